# revision 21
# baseline (speedup 1.0000x reference)
"""ABCLinear distributed Bass kernel for 8 TRN2 NeuronCores.

Computes out = x @ W_eff^T + bias where
  W_eff = W + sum_f tanh(A_f) @ B_f @ C_f
Column-parallel: W, As, bias sharded along out_features across 8 cores;
x, Bs, Cs replicated. Each core computes its [8192, 512] output shard.

Per-core device algorithm (NP = number of fp8 DoubleRow chunk-pairs):
  1. tanhA[(f,r), o] = tanh(AsT shard)                        (ScalarE)
  2. ABT[(f,k), o]   = block-diag(Bs)^T @ tanhA      (1 matmul, float32r)
  3. weffT[i, o]·S   : per 128-row chunk, a correction matmul plus an
                       identity-matmul accumulate W^T into one PSUM tile
                       (ct/wt pre-scaled by S=256 host-side). Chunks
                       0..2NP-1 evict to fp8 e4m3 DoubleRow moving tiles,
                       the rest to bf16, alternating DVE/ACT by chunk.
  4. out[t, o]       = x @ weffT + bias: per 128-token strip, NP fp8
                       DoubleRow matmuls (each contracts 256 rows in the
                       time of one bf16 matmul) + (32-2NP) bf16 matmuls
                       accumulate S·out in PSUM; eviction is one DVE
                       scalar_tensor_tensor: out = psum*(1/S) + bias.

x bf16 + e4m3 chunks prepped host-side. NP=0 reproduces pure-bf16
(~2.8e-3 rel rms); each pair adds sqrt(1/16)*3.5e-2 in quadrature and
saves ~13.8us of PE time.
"""

import numpy as np
import ml_dtypes

import concourse.mybir as mybir
import concourse.tile as tile
from concourse import bacc
from concourse.bass_utils import run_bass_kernel_spmd

T, I, O, R, F = 8192, 4096, 4096, 64, 2
N_CORES = 8
OS = O // N_CORES      # 512 out features per core
TS = 128               # tokens per strip
IC = I // 128          # 32 contraction chunks
KF = F * R             # 128 packed (factor, rank) contraction for W_eff
WG = 4                 # wt chunks per DMA group (1 MiB per DMA)
SCALE = 256.0          # W-path pre-scale (keeps fp8 chunks in normal range;
                       # max|S*Weff| ~ 22, e4m3 max 240)

F32 = mybir.dt.float32
F32R = mybir.dt.float32r
BF16 = mybir.dt.bfloat16
FP8 = mybir.dt.float8e4
DRMODE = mybir.MatmulPerfMode.DoubleRow
E4 = ml_dtypes.float8_e4m3

_CACHE = {}
_IDENT = np.eye(128, dtype=np.float32)


def _build(n_strips, np_pairs=0, xbufs=6, wtbufs=8, psbufs=8):
    np8 = np_pairs
    nbf = IC - 2 * np8     # bf16 chunks (ic = 2*np8 .. 31)
    nc = bacc.Bacc()
    xt = nc.declare_dram_parameter("xt", [n_strips, 128, nbf, TS], BF16,
                                   isOutput=False)
    if np8:
        x8t = nc.declare_dram_parameter("x8t", [n_strips, 128, np8, 2, TS],
                                        FP8, isOutput=False)
    wt = nc.declare_dram_parameter("wt", [IC // WG, 128, WG, OS], BF16,
                                   isOutput=False)
    ident = nc.declare_dram_parameter("ident", [128, 128], BF16, isOutput=False)
    ct = nc.declare_dram_parameter("ct", [KF, IC, 128], BF16, isOutput=False)
    ast = nc.declare_dram_parameter("ast", [KF, OS], F32, isOutput=False)
    bs = nc.declare_dram_parameter("bs", [KF, KF], F32R, isOutput=False)
    bias_full = nc.declare_dram_parameter("bias_full", [128, OS], F32,
                                          isOutput=False)
    out = nc.declare_dram_parameter("out", [n_strips * TS, OS], F32,
                                    isOutput=True)

    with tile.TileContext(nc) as tc:
        with (
            tc.tile_pool(name="resident", bufs=1) as resident,
            tc.tile_pool(name="prolog", bufs=1) as prolog,
            tc.tile_pool(name="wtp", bufs=wtbufs) as wtp,
            tc.tile_pool(name="xp", bufs=xbufs) as xp,
            tc.tile_pool(name="outp", bufs=3) as outp,
            tc.tile_pool(name="psum", bufs=psbufs, space="PSUM") as psum,
        ):
            # ---- prologue DMAs, split across the sync and ACT rings ----
            # sync ring: ct + tanh-chain inputs + ident + wt group 0, then
            # the x strips stream. ACT ring: wt groups 1..7, bias last.
            # Production consumes chunks in order, so group 0 on the fast
            # path unblocks the first strip matmuls early.
            ct_sb = prolog.tile([KF, IC, 128], BF16)
            nc.sync.dma_start(ct_sb[:], ct[:])
            ast_sb = prolog.tile([KF, OS], F32)
            nc.sync.dma_start(ast_sb[:], ast[:])
            bs_sb = prolog.tile([KF, KF], F32R)
            nc.sync.dma_start(bs_sb[:], bs[:])
            ident_sb = prolog.tile([128, 128], BF16)
            nc.sync.dma_start(ident_sb[:], ident[:])

            weff = resident.tile([128, IC - 2 * np8, OS], BF16)
            if np8:
                w8 = resident.tile([128, np8, 2, OS], FP8)
            biasf = resident.tile([128, OS], F32)

            tanh_sb = prolog.tile([KF, OS], F32R)
            nc.scalar.activation(
                tanh_sb[:], ast_sb[:], mybir.ActivationFunctionType.Tanh
            )
            abt_ps = psum.tile([KF, OS], F32, tag="ps", name="abt_ps")
            nc.tensor.matmul(
                abt_ps[:], lhsT=bs_sb[:], rhs=tanh_sb[:], start=True, stop=True
            )
            abt = prolog.tile([KF, OS], BF16)
            nc.vector.tensor_copy(out=abt[:], in_=abt_ps[:])

            # ---- W_eff chunk production ----
            for g in range(IC // WG):
                wt_sb = wtp.tile([128, WG, OS], BF16)
                if g % 2 == 0:
                    nc.sync.dma_start(wt_sb[:], wt[g])
                else:
                    nc.scalar.dma_start(wt_sb[:], wt[g])
                for j in range(WG):
                    ic = g * WG + j
                    wps = psum.tile([128, OS], F32, tag="ps", name=f"wps{ic}")
                    nc.tensor.matmul(
                        wps[:], lhsT=ct_sb[:, ic, :], rhs=abt[:],
                        start=True, stop=False,
                    )
                    nc.tensor.matmul(
                        wps[:], lhsT=ident_sb[:], rhs=wt_sb[:, j, :],
                        start=False, stop=True,
                    )
                    # alternate eviction engines so neither ACT nor DVE
                    # becomes the production-rate bottleneck
                    if ic < 2 * np8:
                        dst = w8[:, ic // 2, ic % 2, :]
                    else:
                        dst = weff[:, ic - 2 * np8, :]
                    if ic % 2 == 0:
                        nc.vector.tensor_copy(out=dst, in_=wps[:])
                    else:
                        nc.scalar.activation(
                            dst, wps[:], mybir.ActivationFunctionType.Copy,
                        )

            nc.scalar.dma_start(biasf[:], bias_full[:])

            # ---- main loop: out strips ----
            for tm in range(n_strips):
                # alternate strips between the two DMA rings to halve the
                # per-ring x latency (out DMAs ride the scalar ring too)
                ring = nc.sync if tm % 2 == 0 else nc.scalar
                x_sb = xp.tile([128, nbf, TS], BF16, tag="x", name=f"x{tm}")
                ring.dma_start(x_sb[:], xt[tm])
                if np8:
                    x8_sb = xp.tile([128, np8, 2, TS], FP8, tag="x8",
                                    name=f"x8_{tm}")
                    ring.dma_start(x8_sb[:], x8t[tm])
                ps = psum.tile([128, OS], F32, tag="ps", name=f"ps{tm}")
                # alternate DR-block position by strip parity so consecutive
                # strips keep the same perf mode across the boundary (halves
                # PE mode transitions)
                n_mm = np8 + nbf
                k = 0

                def dr_block():
                    nonlocal k
                    for i in range(np8):
                        nc.tensor.matmul(
                            ps[:], lhsT=x8_sb[:, i], rhs=w8[:, i],
                            start=(k == 0), stop=(k == n_mm - 1),
                            perf_mode=DRMODE,
                        )
                        k += 1

                def bf_block():
                    nonlocal k
                    for j in range(nbf):
                        ic = 2 * np8 + j
                        nc.tensor.matmul(
                            ps[:], lhsT=x_sb[:, j, :], rhs=weff[:, j, :],
                            start=(k == 0), stop=(k == n_mm - 1),
                        )
                        k += 1

                if tm % 2 == 0:
                    dr_block()
                    bf_block()
                else:
                    bf_block()
                    dr_block()
                o_sb = outp.tile([128, OS], F32)
                nc.vector.scalar_tensor_tensor(
                    out=o_sb[:], in0=ps[:], scalar=1.0 / SCALE, in1=biasf[:],
                    op0=mybir.AluOpType.mult, op1=mybir.AluOpType.add,
                )
                nc.scalar.dma_start(out[tm * TS : (tm + 1) * TS, :], o_sb[:])
    nc.compile()
    return nc


def _get_nc(n_strips, **opts):
    key = (n_strips, tuple(sorted(opts.items())))
    if key not in _CACHE:
        _CACHE[key] = _build(n_strips, **opts)
    return _CACHE[key]


def _prepare_inputs(x, W, bias, As, Bs, Cs, n_strips, np_pairs=0):
    np8 = np_pairs
    nbf = IC - 2 * np8
    t_used = n_strips * TS
    # x strips: [tm][p, c, n] = x[tm*TS + n, c*128 + p]
    xr = x[:t_used].reshape(n_strips, TS, IC, 128).transpose(0, 3, 2, 1)
    xt = np.ascontiguousarray(xr[:, :, 2 * np8:, :]).astype(ml_dtypes.bfloat16)
    x8t = None
    if np8:
        # [tm][p, i, s, n] = x[tm*TS + n, (2i+s)*128 + p]
        x8t = np.ascontiguousarray(
            xr[:, :, : 2 * np8, :].reshape(n_strips, 128, np8, 2, TS)
        ).astype(E4)
    ct = np.ascontiguousarray(Cs.reshape(KF, IC, 128) * SCALE).astype(
        ml_dtypes.bfloat16)
    # block-diagonal packed B: bs[(f,r'), (f,k)] = Bs[f, r', k]
    bsm = np.zeros((KF, KF), dtype=np.float32)
    for f in range(F):
        bsm[f * R : (f + 1) * R, f * R : (f + 1) * R] = Bs[f]
    ident = _IDENT.astype(ml_dtypes.bfloat16)
    in_maps = []
    for c in range(N_CORES):
        Ws = W[c * OS : (c + 1) * OS]                  # [OS, I]
        wtc = np.ascontiguousarray(
            (Ws.T * SCALE).reshape(IC // WG, WG, 128, OS).transpose(0, 2, 1, 3)
        ).astype(ml_dtypes.bfloat16)
        As_c = As[:, c * OS : (c + 1) * OS, :]         # [F, OS, R]
        astc = np.ascontiguousarray(As_c.transpose(0, 2, 1).reshape(KF, OS))
        bias_fullc = np.ascontiguousarray(
            np.broadcast_to(bias[c * OS : (c + 1) * OS][None, :], (128, OS))
        )
        m = {"xt": xt, "wt": wtc, "ct": ct, "ast": astc, "bs": bsm,
             "bias_full": bias_fullc, "ident": ident}
        if np8:
            m["x8t"] = x8t
        in_maps.append(m)
    return in_maps


def _run(x, W, bias, As, Bs, Cs, n_strips, trace=False, **opts):
    nc = _get_nc(n_strips, **opts)
    in_maps = _prepare_inputs(x, W, bias, As, Bs, Cs, n_strips,
                              np_pairs=opts.get("np_pairs", 0))
    res = run_bass_kernel_spmd(nc, in_maps, core_ids=list(range(N_CORES)),
                               trace=trace)
    full = np.concatenate([res.results[c]["out"] for c in range(N_CORES)], axis=1)
    return full, res


_BEST = dict(np_pairs=5)


def kernel(x, W, bias, As, Bs, Cs):
    x = np.asarray(x, dtype=np.float32)
    W = np.asarray(W, dtype=np.float32)
    bias = np.asarray(bias, dtype=np.float32)
    As = np.asarray(As, dtype=np.float32)
    Bs = np.asarray(Bs, dtype=np.float32)
    Cs = np.asarray(Cs, dtype=np.float32)
    full, _ = _run(x, W, bias, As, Bs, Cs, T // TS, **_BEST)
    return full


# revision 25
# speedup vs baseline: 1.0091x; 1.0091x over previous
"""ABCLinear distributed Bass kernel for 8 TRN2 NeuronCores.

Computes out = x @ W_eff^T + bias where
  W_eff = W + sum_f tanh(A_f) @ B_f @ C_f
Column-parallel: W, As, bias sharded along out_features across 8 cores;
x, Bs, Cs replicated. Each core computes its [8192, 512] output shard.

Per-core device algorithm (NP = number of fp8 DoubleRow chunk-pairs):
  1. tanhA[(f,r), o] = tanh(AsT shard)                        (ScalarE)
  2. ABT[(f,k), o]   = block-diag(Bs)^T @ tanhA      (1 matmul, float32r)
  3. weffT[i, o]·S   : per 128-row chunk, a correction matmul plus an
                       identity-matmul accumulate W^T into one PSUM tile
                       (ct/wt pre-scaled by S=256 host-side). Chunks
                       0..2NP-1 evict to fp8 e4m3 DoubleRow moving tiles,
                       the rest to bf16, alternating DVE/ACT by chunk.
  4. out[t, o]       = x @ weffT + bias: per 128-token strip, NP fp8
                       DoubleRow matmuls (each contracts 256 rows in the
                       time of one bf16 matmul) + (32-2NP) bf16 matmuls
                       accumulate S·out in PSUM; eviction is one DVE
                       scalar_tensor_tensor: out = psum*(1/S) + bias.

x bf16 + e4m3 chunks prepped host-side. NP=0 reproduces pure-bf16
(~2.8e-3 rel rms); each pair adds sqrt(1/16)*3.5e-2 in quadrature and
saves ~13.8us of PE time.
"""

import numpy as np
import ml_dtypes

import concourse.mybir as mybir
import concourse.tile as tile
from concourse import bacc
from concourse.bass_utils import run_bass_kernel_spmd

T, I, O, R, F = 8192, 4096, 4096, 64, 2
N_CORES = 8
OS = O // N_CORES      # 512 out features per core
TS = 128               # tokens per strip
IC = I // 128          # 32 contraction chunks
KF = F * R             # 128 packed (factor, rank) contraction for W_eff
WG = 4                 # wt chunks per DMA group (1 MiB per DMA)
SCALE = 256.0          # W-path pre-scale (keeps fp8 chunks in normal range;
                       # max|S*Weff| ~ 22, e4m3 max 240)

F32 = mybir.dt.float32
F32R = mybir.dt.float32r
BF16 = mybir.dt.bfloat16
FP8 = mybir.dt.float8e4
DRMODE = mybir.MatmulPerfMode.DoubleRow
E4 = ml_dtypes.float8_e4m3

_CACHE = {}
_IDENT = np.eye(128, dtype=np.float32)


def _build(n_strips, np_pairs=0, xbufs=6, wtbufs=8, psbufs=8):
    np8 = np_pairs
    nbf = IC - 2 * np8     # bf16 chunks (ic = 2*np8 .. 31)
    nc = bacc.Bacc()
    xt = nc.declare_dram_parameter("xt", [n_strips, 128, nbf, TS], BF16,
                                   isOutput=False)
    if np8:
        x8t = nc.declare_dram_parameter("x8t", [n_strips, 128, np8, 2, TS],
                                        FP8, isOutput=False)
    wt = nc.declare_dram_parameter("wt", [IC // WG, 128, WG, OS], BF16,
                                   isOutput=False)
    ident = nc.declare_dram_parameter("ident", [128, 128], BF16, isOutput=False)
    ct = nc.declare_dram_parameter("ct", [KF, IC, 128], BF16, isOutput=False)
    ast = nc.declare_dram_parameter("ast", [KF, OS], F32, isOutput=False)
    bs = nc.declare_dram_parameter("bs", [KF, KF], F32R, isOutput=False)
    bias_full = nc.declare_dram_parameter("bias_full", [128, OS], F32,
                                          isOutput=False)
    out = nc.declare_dram_parameter("out", [n_strips * TS, OS], BF16,
                                    isOutput=True)

    with tile.TileContext(nc) as tc:
        with (
            tc.tile_pool(name="resident", bufs=1) as resident,
            tc.tile_pool(name="prolog", bufs=1) as prolog,
            tc.tile_pool(name="wtp", bufs=wtbufs) as wtp,
            tc.tile_pool(name="xp", bufs=xbufs) as xp,
            tc.tile_pool(name="outp", bufs=3) as outp,
            tc.tile_pool(name="psum", bufs=psbufs, space="PSUM") as psum,
        ):
            # ---- prologue DMAs, split across the sync and ACT rings ----
            # sync ring: ct + tanh-chain inputs + ident + wt group 0, then
            # the x strips stream. ACT ring: wt groups 1..7, bias last.
            # Production consumes chunks in order, so group 0 on the fast
            # path unblocks the first strip matmuls early.
            # PE warm-up: dummy matmuls on memset data occupy the otherwise
            # idle PE bootstrap window so the HAM clock gate releases
            # (1.2 -> 2.4 GHz) earlier into the real matmul stream
            wsrc = prolog.tile([128, OS], BF16)
            nc.vector.memset(wsrc[:], 0.0)
            warm_ps = psum.tile([128, OS], F32, tag="ps", name="warm_ps")
            for _ in range(3):
                nc.tensor.matmul(warm_ps[:], lhsT=wsrc[:, :128], rhs=wsrc[:],
                                 start=True, stop=True)

            ct_sb = prolog.tile([KF, IC, 128], BF16)
            nc.sync.dma_start(ct_sb[:], ct[:])
            ast_sb = prolog.tile([KF, OS], F32)
            nc.sync.dma_start(ast_sb[:], ast[:])
            bs_sb = prolog.tile([KF, KF], F32R)
            nc.sync.dma_start(bs_sb[:], bs[:])
            ident_sb = prolog.tile([128, 128], BF16)
            nc.sync.dma_start(ident_sb[:], ident[:])

            weff = resident.tile([128, IC - 2 * np8, OS], BF16)
            if np8:
                w8 = resident.tile([128, np8, 2, OS], FP8)
            biasf = resident.tile([128, OS], F32)

            tanh_sb = prolog.tile([KF, OS], F32R)
            nc.scalar.activation(
                tanh_sb[:], ast_sb[:], mybir.ActivationFunctionType.Tanh
            )
            abt_ps = psum.tile([KF, OS], F32, tag="ps", name="abt_ps")
            nc.tensor.matmul(
                abt_ps[:], lhsT=bs_sb[:], rhs=tanh_sb[:], start=True, stop=True
            )
            abt = prolog.tile([KF, OS], BF16)
            nc.vector.tensor_copy(out=abt[:], in_=abt_ps[:])

            # ---- W_eff chunk production ----
            for g in range(IC // WG):
                wt_sb = wtp.tile([128, WG, OS], BF16)
                if g % 2 == 0:
                    nc.sync.dma_start(wt_sb[:], wt[g])
                else:
                    nc.scalar.dma_start(wt_sb[:], wt[g])
                for j in range(WG):
                    ic = g * WG + j
                    wps = psum.tile([128, OS], F32, tag="ps", name=f"wps{ic}")
                    nc.tensor.matmul(
                        wps[:], lhsT=ct_sb[:, ic, :], rhs=abt[:],
                        start=True, stop=False,
                    )
                    nc.tensor.matmul(
                        wps[:], lhsT=ident_sb[:], rhs=wt_sb[:, j, :],
                        start=False, stop=True,
                    )
                    # alternate eviction engines so neither ACT nor DVE
                    # becomes the production-rate bottleneck
                    if ic < 2 * np8:
                        dst = w8[:, ic // 2, ic % 2, :]
                    else:
                        dst = weff[:, ic - 2 * np8, :]
                    if ic % 2 == 0:
                        nc.vector.tensor_copy(out=dst, in_=wps[:])
                    else:
                        nc.scalar.activation(
                            dst, wps[:], mybir.ActivationFunctionType.Copy,
                        )

            nc.scalar.dma_start(biasf[:], bias_full[:])

            # ---- main loop: out strips ----
            for tm in range(n_strips):
                # alternate strips between the two DMA rings to halve the
                # per-ring x latency (out DMAs ride the scalar ring too)
                ring = nc.sync if tm % 2 == 0 else nc.scalar
                x_sb = xp.tile([128, nbf, TS], BF16, tag="x", name=f"x{tm}")
                ring.dma_start(x_sb[:], xt[tm])
                if np8:
                    x8_sb = xp.tile([128, np8, 2, TS], FP8, tag="x8",
                                    name=f"x8_{tm}")
                    ring.dma_start(x8_sb[:], x8t[tm])
                ps = psum.tile([128, OS], F32, tag="ps", name=f"ps{tm}")
                # alternate DR-block position by strip parity so consecutive
                # strips keep the same perf mode across the boundary (halves
                # PE mode transitions)
                n_mm = np8 + nbf
                k = 0

                def dr_block():
                    nonlocal k
                    for i in range(np8):
                        nc.tensor.matmul(
                            ps[:], lhsT=x8_sb[:, i], rhs=w8[:, i],
                            start=(k == 0), stop=(k == n_mm - 1),
                            perf_mode=DRMODE,
                        )
                        k += 1

                def bf_block():
                    nonlocal k
                    for j in range(nbf):
                        ic = 2 * np8 + j
                        nc.tensor.matmul(
                            ps[:], lhsT=x_sb[:, j, :], rhs=weff[:, j, :],
                            start=(k == 0), stop=(k == n_mm - 1),
                        )
                        k += 1

                if tm % 2 == 0:
                    dr_block()
                    bf_block()
                else:
                    bf_block()
                    dr_block()
                o_sb = outp.tile([128, OS], BF16)
                nc.vector.scalar_tensor_tensor(
                    out=o_sb[:], in0=ps[:], scalar=1.0 / SCALE, in1=biasf[:],
                    op0=mybir.AluOpType.mult, op1=mybir.AluOpType.add,
                )
                nc.scalar.dma_start(out[tm * TS : (tm + 1) * TS, :], o_sb[:])
    nc.compile()
    return nc


def _get_nc(n_strips, **opts):
    key = (n_strips, tuple(sorted(opts.items())))
    if key not in _CACHE:
        _CACHE[key] = _build(n_strips, **opts)
    return _CACHE[key]


def _prepare_inputs(x, W, bias, As, Bs, Cs, n_strips, np_pairs=0):
    np8 = np_pairs
    nbf = IC - 2 * np8
    t_used = n_strips * TS
    # x strips: [tm][p, c, n] = x[tm*TS + n, c*128 + p]
    xr = x[:t_used].reshape(n_strips, TS, IC, 128).transpose(0, 3, 2, 1)
    xt = np.ascontiguousarray(xr[:, :, 2 * np8:, :]).astype(ml_dtypes.bfloat16)
    x8t = None
    if np8:
        # [tm][p, i, s, n] = x[tm*TS + n, (2i+s)*128 + p]
        x8t = np.ascontiguousarray(
            xr[:, :, : 2 * np8, :].reshape(n_strips, 128, np8, 2, TS)
        ).astype(E4)
    ct = np.ascontiguousarray(Cs.reshape(KF, IC, 128) * SCALE).astype(
        ml_dtypes.bfloat16)
    # block-diagonal packed B: bs[(f,r'), (f,k)] = Bs[f, r', k]
    bsm = np.zeros((KF, KF), dtype=np.float32)
    for f in range(F):
        bsm[f * R : (f + 1) * R, f * R : (f + 1) * R] = Bs[f]
    ident = _IDENT.astype(ml_dtypes.bfloat16)
    in_maps = []
    for c in range(N_CORES):
        Ws = W[c * OS : (c + 1) * OS]                  # [OS, I]
        wtc = np.ascontiguousarray(
            (Ws.T * SCALE).reshape(IC // WG, WG, 128, OS).transpose(0, 2, 1, 3)
        ).astype(ml_dtypes.bfloat16)
        As_c = As[:, c * OS : (c + 1) * OS, :]         # [F, OS, R]
        astc = np.ascontiguousarray(As_c.transpose(0, 2, 1).reshape(KF, OS))
        bias_fullc = np.ascontiguousarray(
            np.broadcast_to(bias[c * OS : (c + 1) * OS][None, :], (128, OS))
        )
        m = {"xt": xt, "wt": wtc, "ct": ct, "ast": astc, "bs": bsm,
             "bias_full": bias_fullc, "ident": ident}
        if np8:
            m["x8t"] = x8t
        in_maps.append(m)
    return in_maps


def _run(x, W, bias, As, Bs, Cs, n_strips, trace=False, **opts):
    nc = _get_nc(n_strips, **opts)
    in_maps = _prepare_inputs(x, W, bias, As, Bs, Cs, n_strips,
                              np_pairs=opts.get("np_pairs", 0))
    res = run_bass_kernel_spmd(nc, in_maps, core_ids=list(range(N_CORES)),
                               trace=trace)
    full = np.concatenate(
        [res.results[c]["out"].astype(np.float32) for c in range(N_CORES)],
        axis=1)
    return full, res


_BEST = dict(np_pairs=5)


def kernel(x, W, bias, As, Bs, Cs):
    x = np.asarray(x, dtype=np.float32)
    W = np.asarray(W, dtype=np.float32)
    bias = np.asarray(bias, dtype=np.float32)
    As = np.asarray(As, dtype=np.float32)
    Bs = np.asarray(Bs, dtype=np.float32)
    Cs = np.asarray(Cs, dtype=np.float32)
    full, _ = _run(x, W, bias, As, Bs, Cs, T // TS, **_BEST)
    return full
